# revision 35
# baseline (speedup 1.0000x reference)
# Trainium2 Bass kernel for nn_AutoRegressive (LSTM warmup + autoregressive decode).
#
# Problem: B=512, T=128, F=64, UNITS=1024, OUT_STEPS=32.
#   warmup: 128 sequential LSTM steps over inputs, keep final (h, c)
#   decode: pred = h @ Wd + bd, feed pred back as x for 31 more steps
#   output: [B, 32, F]
#
# Strategy: pure 8-way data parallelism on the batch axis (64 rows/core),
# weights replicated, zero cross-core communication. Per step the dominant
# matmul z = x @ Wk + h @ Wr is computed with h^T-stationary matmuls
# (lhsT = h^T[k-chunk] [128, 64]) streaming Wr columns. Because the local
# batch is 64 (< 128 array columns), each matmul pair is column-tiled at
# (0,0)/(0,64) covering the lo/hi unit-halves of a gate concurrently.
#
# Restructure vs the 1.85ms baseline (measured 1.51ms same-conditions vs
# the baseline's 2.17ms; 9.1us/warm step, 9.7us/decode step, PE ~100%):
#  - Per step the gates are emitted as column runs in completion order
#    f(512), i(512), g0, o0, g1, o1 (256 each): x@Wk opens each bank's
#    accumulation group, the 8-chunk h@Wr k-loop fills it. Gate slices
#    complete progressively through the PE stream so the sigmoid/tanh/
#    cell chain runs concurrently with the matmuls instead of serially
#    at the step tail (was ~3.4us of tail). o0 closes before g1 so the
#    slice-0 h (-> pair-0 transpose + hT copy of the next step) is ready
#    mid-stream, and h1 lands just before the pair-1 transpose needs it.
#    In decode, pred (and xd) is computed right after the transposes and
#    each gate gets [h@Wr opener; x@Wk closer] back to back - closing all
#    groups at stream end instead serializes the whole activation chain
#    after it (+4.3us/decode step measured).
#  - PSUM rule learned the hard way (and verified in CoreSim): start=True
#    zero-marks the whole 2KB bank row per written partition, so a bank
#    may hold only ONE open accumulation group at a time. f and i are
#    single 512-wide groups in their banks; g/o pairs live in separate
#    banks. CoreSim's group checker is partition-unaware, so the
#    partition-64:128 twin of each opener sets skip_group_check.
#  - Gate/cell elementwise chain in fp16 (c state, sigmoids, tanh) and
#    bf16 (h): 2-byte dtypes + all-SBUF operands put the DVE in its 4x
#    mode (~510ns -> ~130ns per [128,256] op).
#  - Engines are FIFO: ACT/DVE ops are emitted in exact readiness order
#    (sig_f0..sig_o1 / c-chain) so nothing blocks the h tail.
#  - The next step's f/i/g x@Wk runs are emitted before the transposes
#    and its o x@Wk between the two transpose pairs, giving the PE work
#    while the previous step's h tail completes (the o banks' previous
#    groups are read by then; their x@Wk must not execute near the
#    boundary of the PREVIOUS step, which is also why they are not
#    hoisted further).
#  - Filler matmuls dropped: PE gaps are now well under the ~3.4us HAM
#    re-throttle window.
# Bias b is folded into an augmented ones-row of x / extra row of Wk.
# pred copies run on ScalarE with bd folded in as an Identity bias.
import os
import sys

sys.path.insert(0, "/opt/trn_rl_repo")

import numpy as np
import ml_dtypes

import concourse.bass as bass
import concourse.mybir as mybir
import concourse.tile as tile
from concourse import bacc
from concourse.bass_utils import run_bass_kernel_spmd
from concourse.masks import make_identity
from contextlib import ExitStack

F32, F16, BF16 = mybir.dt.float32, mybir.dt.float16, mybir.dt.bfloat16
AF = mybir.ActivationFunctionType
Alu = mybir.AluOpType

B_FULL, T_FULL, F_DIM, UNITS = 512, 128, 64, 1024
N_CORES = 8
B = B_FULL // N_CORES          # 64 local batch rows
NK = UNITS // 128              # 8 k-chunks of the recurrent contraction
GATES = [1, 0, 2, 3]           # processing order f,i,g,o (orig packing i,f,c,o)
HT_ORDER = (0, 4, 1, 5, 2, 6, 3, 7)   # k-chunk consumption order

_NC_CACHE = {}


def _build(n_warm: int, n_out: int):
    """Build the per-core Bass program. n_out = number of predictions (32)."""
    key = (n_warm, n_out)
    if key in _NC_CACHE:
        return _NC_CACHE[key]

    n_dec = n_out - 1  # LSTM steps in decode phase

    nc = bacc.Bacc("TRN2", target_bir_lowering=False, debug=False,
                   num_devices=N_CORES)
    xt_ext = nc.dram_tensor("xt", [n_warm, F_DIM + 1, B], BF16,
                            kind="ExternalInput")
    wr_ext = nc.dram_tensor("wr", [128, NK * 8 * 512], BF16,
                            kind="ExternalInput")
    wk_ext = nc.dram_tensor("wk", [F_DIM + 1, 8 * 512], BF16,
                            kind="ExternalInput")
    wd_ext = nc.dram_tensor("wd", [128, NK * F_DIM], BF16,
                            kind="ExternalInput")
    bd_ext = nc.dram_tensor("bd", [F_DIM, 1], F32, kind="ExternalInput")
    out_ext = nc.dram_tensor("out", [F_DIM, n_out * B], F32,
                             kind="ExternalOutput")

    with ExitStack() as ctx:
        tc = ctx.enter_context(tile.TileContext(nc))
        wpool = ctx.enter_context(tc.tile_pool(name="w", bufs=1))
        xpool = ctx.enter_context(tc.tile_pool(name="x", bufs=3))
        hTpool = ctx.enter_context(tc.tile_pool(name="hT", bufs=2))
        hpool = ctx.enter_context(tc.tile_pool(name="h", bufs=2))
        cpool = ctx.enter_context(tc.tile_pool(name="c", bufs=2))
        gpool = ctx.enter_context(tc.tile_pool(name="g", bufs=2))
        zpool = ctx.enter_context(tc.tile_pool(name="z", bufs=1, space="PSUM"))
        zspool = ctx.enter_context(tc.tile_pool(name="zs", bufs=2, space="PSUM"))
        tpool = ctx.enter_context(tc.tile_pool(name="tp", bufs=1, space="PSUM"))

        # Staging order matters for the prologue: t=0 needs only wk (0.5MB),
        # so it goes first and the 8MB wr transfer is split into per-chunk
        # slices issued in k-consumption order — warmup starts ~20us
        # earlier and chunk arrivals pipeline against the first steps'
        # k-loops (each h@Wr matmul depends only on its chunk's slice).
        # One dma_start lands on ~one DMA engine (~18GB/s), so the 0.5MB
        # wk as a single transfer kept the first x@Wk waiting ~30us. Eight
        # 64KB block slices in t=0's consumption order parallelize across
        # engines -> first PE op at ~4us.
        wk_sb = wpool.tile([F_DIM + 1, 8 * 512], BF16)
        for blk in (0, 4, 1, 5, 2, 6, 3, 7):
            nc.sync.dma_start(wk_sb[:, blk * 512:(blk + 1) * 512],
                              wk_ext[:, blk * 512:(blk + 1) * 512])
        wd_sb = wpool.tile([128, NK * F_DIM], BF16)
        nc.sync.dma_start(wd_sb[:], wd_ext[:])
        bd_sb = wpool.tile([F_DIM, 1], F32)
        nc.sync.dma_start(bd_sb[:], bd_ext[:])
        wr_sb = wpool.tile([128, NK * 8 * 512], BF16)
        for k in HT_ORDER:
            nc.sync.dma_start(wr_sb[:, k * 4096:(k + 1) * 4096],
                              wr_ext[:, k * 4096:(k + 1) * 4096])
        identb = wpool.tile([128, 128], BF16)
        make_identity(nc, identb[:])
        preds_sb = wpool.tile([F_DIM, n_out * B], F32)
        xd_sb = wpool.tile([F_DIM + 1, B], BF16)
        nc.vector.memset(xd_sb[F_DIM:F_DIM + 1, :], 1.0)

        state = {"h0": None, "h1": None, "c": None}
        # hT column layout: transpose of h[:, j*128:(j+1)*128] yields unit
        # chunks j (cols 0:64) and j+4 (cols 64:128); store them adjacently
        # so each transpose pair needs ONE contiguous DVE copy.
        HT_POS = {}
        for j in range(4):
            HT_POS[j] = 2 * j
            HT_POS[j + 4] = 2 * j + 1

        def hT_sl(k):
            p = HT_POS[k]
            t = state["hTa"] if p < 4 else state["hTb"]
            return t[:, (p % 4) * B:(p % 4 + 1) * B]

        def transpose_pair(half):
            """h half (bf16, batch-major split layout) -> hT chunks (bf16).
            half 0: chunks 0,4,1,5 from h0; half 1: chunks 2,6,3,7 from h1.
            Each pair gets its own PSUM bank and its own hT SBUF tile."""
            js = (0, 1) if half == 0 else (2, 3)
            tag = "tp0" if half == 0 else "tps"
            tps = tpool.tile([128, 1024], BF16, name=tag, tag=tag)[:, 0:256]
            hT = hTpool.tile([128, 4 * B], BF16,
                             name="hTa" if half == 0 else "hTb",
                             tag="hTa" if half == 0 else "hTb")
            state["hTa" if half == 0 else "hTb"] = hT
            for jj, j in enumerate(js):
                h_half = state["h0"] if j < 2 else state["h1"]
                nc.tensor.transpose(tps[:, jj * 128:(jj + 1) * 128],
                                    h_half[:, (j % 2) * 128:(j % 2 + 1) * 128],
                                    identb[:])
            nc.vector.tensor_copy(hT[:], tps[:])

        def alloc_z():
            """Gate z PSUM tiles, processing order f, i, g, o. f and i are
            full [128,512] banks; g and o are two half-used banks each (a
            shared bank would serialize reads against the bank-mate's
            writes)."""
            zf = zpool.tile([128, 512], F32, name="zf", tag="zf")
            zi = zpool.tile([128, 512], F32, name="zi", tag="zi")
            zg = [zspool.tile([128, 512], F32, name="zg%d" % s, tag="zg")[:, 0:256]
                  for s in (0, 1)]
            zo = [zspool.tile([128, 512], F32, name="zo%d" % s, tag="zo")[:, 0:256]
                  for s in (0, 1)]
            return (zf, zi, zg, zo)

        def runs_of(zs):
            """Six runs: (blk, psum_region, col_lo_in_block, width). f and i
            are single 512-wide runs (one PSUM accumulation group per bank —
            start=True zero-marks the whole 2KB bank row, so a bank must
            never hold two open groups); g and o are 256-wide in their own
            banks. blk = processing-order gate (0=f 1=i 2=g 3=o)."""
            zf, zi, zg, zo = zs
            return (
                (0, zf[:, 0:512], 0, 512),
                (1, zi[:, 0:512], 0, 512),
                (2, zg[0], 0, 256), (2, zg[1], 256, 256),
                (3, zo[0], 0, 256), (3, zo[1], 256, 256),
            )

        def emit_xwk_run(run, x_sb, start, stop):
            """x @ Wk (+b) pair for one run. skip_group_check on the half-1
            opener: CoreSim's zero-region group view is partition-unaware
            and false-positives on the second (partition 64:128) opener of
            a bank; on HW the two halves zero disjoint partition rows."""
            blk, z, lo, w = run
            for half in (0, 1):
                o = (half * 4 + blk) * 512 + lo
                nc.tensor.matmul(z[half * 64:(half + 1) * 64, :],
                                 x_sb[:], wk_sb[:, o:o + w],
                                 start=start, stop=stop,
                                 skip_group_check=(half == 1))

        def emit_hwr_run(run, kis, start_at_first, stop_at_last):
            """h @ Wr chunk-pairs for one run, chunks kis (actual k values)."""
            blk, z, lo, w = run
            for idx, k in enumerate(kis):
                for half in (0, 1):
                    start = start_at_first and idx == 0
                    stop = stop_at_last and idx == len(kis) - 1
                    o = (k * 8 + half * 4 + blk) * 512 + lo
                    nc.tensor.matmul(
                        z[half * 64:(half + 1) * 64, :],
                        hT_sl(k), wr_sb[:, o:o + w],
                        start=start, stop=stop,
                        skip_group_check=(half == 1))

        def pred_block(d):
            """pred_d^T = Wd^T @ h + bd from current hT; returns x_dec tile."""
            # shares the pair-0 transpose bank (released right after copy-a)
            pp = tpool.tile([F_DIM, 512], F32, name="pp", tag="tp0")[:, 0:B]
            for ki, k in enumerate(HT_ORDER):
                nc.tensor.matmul(pp[:], wd_sb[:, k * F_DIM:(k + 1) * F_DIM],
                                 hT_sl(k), start=(ki == 0), stop=(ki == 7))
            # Copies on ScalarE (off the DVE queue); bd is per-partition on
            # pred^T so it folds into the copy as an Identity bias. The xd
            # copy goes first: it gates the decode x@Wk matmuls.
            ret = None
            if d < n_out - 1:
                nc.scalar.activation(xd_sb[0:F_DIM, :], pp[:],
                                     AF.Identity, bias=bd_sb[:])
                ret = xd_sb
            nc.scalar.activation(preds_sb[:, d * B:(d + 1) * B], pp[:],
                                 AF.Identity, bias=bd_sb[:])
            return ret

        def gate_chain(zs, first):
            """ACT/DVE emission in engine-FIFO readiness order. fp16 gates
            and cell state (DVE 4x mode), bf16 h (PE operand)."""
            zf, zi, zg, zo = zs
            c_prev = state["c"]
            sig_f, sig_i, tanh_g, sig_o, tanh_c = [], [], [], [], []
            cs, hs, t1s = [None, None], [None, None], [None, None]

            def act(dst_list, src, func, s, tag):
                t = gpool.tile([128, 256], F16, tag="%s%d" % (tag, s),
                               name=tag)
                nc.scalar.activation(t[:], src, func)
                dst_list.append(t)

            if not first:
                # sig_f0, sig_f1 then the two c-mults (DVE) run early
                act(sig_f, zf[:, 0:256], AF.Sigmoid, 0, "sf")
                act(sig_f, zf[:, 256:512], AF.Sigmoid, 1, "sf")
                for s in (0, 1):
                    cs[s] = cpool.tile([128, 256], F16, tag="c%d" % s, name="c")
                    nc.vector.tensor_tensor(cs[s][:], sig_f[s][:],
                                            c_prev[s][:], Alu.mult)
            act(sig_i, zi[:, 0:256], AF.Sigmoid, 0, "si")
            act(sig_i, zi[:, 256:512], AF.Sigmoid, 1, "si")

            def c_update(s):
                if first:
                    cs[s] = cpool.tile([128, 256], F16, tag="c%d" % s, name="c")
                    nc.vector.tensor_tensor(cs[s][:], sig_i[s][:],
                                            tanh_g[s][:], Alu.mult)
                else:
                    t1s[s] = gpool.tile([128, 256], F16, tag="t1_%d" % s,
                                        name="t1")
                    nc.vector.tensor_tensor(t1s[s][:], sig_i[s][:],
                                            tanh_g[s][:], Alu.mult)
                    nc.vector.tensor_tensor(cs[s][:], cs[s][:], t1s[s][:],
                                            Alu.add)

            # ACT/DVE emission follows input-readiness order for the
            # f,i,g0,o0,g1,o1 h@Wr run order: the slice-0 chain (tanh_g0 ->
            # c0 -> tanh_c0 -> sig_o0 -> h0) completes mid-stream so the
            # pair-0 transpose + hT copy of the next step fire early, and
            # h1 lands just before the pair-1 transpose needs it.
            def slice_tail(s, so_first=False):
                act(tanh_g, zg[s][:], AF.Tanh, s, "tg")
                c_update(s)
                # slice 1: sig_o before tanh_c — zo1 closes the stream, so
                # its read (which the next boundary's o x@Wk WARs on) must
                # not queue behind the c1 chain's tanh.
                if so_first:
                    act(sig_o, zo[s][:], AF.Sigmoid, s, "so")
                    act(tanh_c, cs[s][:], AF.Tanh, s, "tc")
                else:
                    act(tanh_c, cs[s][:], AF.Tanh, s, "tc")
                    act(sig_o, zo[s][:], AF.Sigmoid, s, "so")
                h = hpool.tile([128, 256], BF16, tag="h%d" % s, name="h")
                nc.vector.tensor_tensor(h[:], sig_o[s][:], tanh_c[s][:],
                                        Alu.mult)
                hs[s] = h

            slice_tail(0)
            slice_tail(1, so_first=True)
            state["h0"], state["h1"], state["c"] = hs[0], hs[1], cs

        # ---- warmup ----
        for t in range(n_warm):
            x_sb = xpool.tile([F_DIM + 1, B], BF16)
            nc.sync.dma_start(x_sb[:], xt_ext[t])
            zs = alloc_z()
            runs = runs_of(zs)
            if t == 0:
                for run in runs:
                    emit_xwk_run(run, x_sb, start=True, stop=True)
            else:
                # f/i/g x@Wk runs first (group openers, no hT dependency):
                # PE work covering the previous step's h tail + transposes.
                # The o runs' PSUM buffers are read (sig_o) only at the very
                # end of the previous step, so their x@Wk (whose start=True
                # zero-marks the region) must execute well after the
                # boundary — emit them after the g h@Wr runs.
                for run in runs[:4]:
                    emit_xwk_run(run, x_sb, start=True, stop=False)
                transpose_pair(0)
                # o-gate x@Wk between the transpose pairs: fills the PE
                # wait for h1 (the pair-1 transpose input). Group-legal:
                # zo0/zo1 are their own banks, opened exactly once; the
                # WAR on the previous step's sig_o reads is past by now.
                for run in runs[4:]:
                    emit_xwk_run(run, x_sb, start=True, stop=False)
                transpose_pair(1)
                # h@Wr run order f,i,g0,o0,g1,o1: zo0 closes before zg1 so
                # sig_o0/h0 complete mid-stream (the scheduler orders the
                # ACT queue by readiness; with o0 last the slice-0 h sat
                # behind the whole slice-1 c chain).
                for ri in (0, 1, 2, 4, 3, 5):
                    emit_hwr_run(runs[ri], HT_ORDER, False, True)
            gate_chain(zs, first=(t == 0))

        # ---- decode: pred (and thus xd) is computed right after the
        # transposes; then per gate [h@Wr opens the group; x@Wk closes it]
        # so the gates complete progressively through the stream exactly
        # like in warmup (closing every group at stream end would serialize
        # the whole activation chain after it: +4.3us/step measured) ----
        for d in range(n_dec):
            zs = alloc_z()
            runs = runs_of(zs)
            transpose_pair(0)
            transpose_pair(1)
            xd = pred_block(d)
            for ri in (0, 1, 2, 4, 3, 5):
                emit_hwr_run(runs[ri], HT_ORDER, True, False)
                emit_xwk_run(runs[ri], xd, start=False, stop=True)
            gate_chain(zs, first=False)
        transpose_pair(0)
        transpose_pair(1)
        pred_block(n_out - 1)

        nc.sync.dma_start(out_ext[:], preds_sb[:])

    nc.finalize()
    _NC_CACHE[key] = nc
    return nc


def _prep_core_inputs(inputs, Wk, Wr, b, Wd, bd, n_warm, n_out):
    """Host-side reshaping/sharding. Returns list of 8 input dicts."""
    bf = lambda a: np.ascontiguousarray(a).astype(ml_dtypes.bfloat16)
    perm = np.array([g * UNITS + hh * 512 + k
                     for hh in (0, 1) for g in GATES for k in range(512)])
    Wk_aug = np.concatenate([Wk, b[None, :]], 0)[:, perm]        # [65, 4096]
    Wr_p = Wr[:, perm]                                           # [1024, 4096]
    wr_dev = bf(np.stack([Wr_p[k * 128:(k + 1) * 128] for k in range(NK)],
                         1).reshape(128, -1))
    wk_dev = bf(Wk_aug)
    wd_dev = bf(np.stack([Wd[k * 128:(k + 1) * 128] for k in range(NK)],
                         1).reshape(128, -1))
    bd_dev = np.ascontiguousarray(bd[:, None]).astype(np.float32)

    in_maps = []
    for c in range(N_CORES):
        xs = inputs[c * B:(c + 1) * B, :n_warm]                  # [64, T, F]
        xt = xs.transpose(1, 2, 0)                               # [T, F, 64]
        xt_aug = np.concatenate(
            [xt, np.ones((n_warm, 1, B), np.float32)], 1)        # [T, 65, 64]
        in_maps.append({
            "xt": bf(xt_aug), "wr": wr_dev, "wk": wk_dev,
            "wd": wd_dev, "bd": bd_dev,
        })
    return in_maps


def kernel(inputs, Wk, Wr, b, Wd, bd, out_steps):
    inputs = np.asarray(inputs, np.float32)
    Wk = np.asarray(Wk, np.float32)
    Wr = np.asarray(Wr, np.float32)
    b = np.asarray(b, np.float32)
    Wd = np.asarray(Wd, np.float32)
    bd = np.asarray(bd, np.float32)
    n_out = int(out_steps)
    n_warm = inputs.shape[1]

    nc = _build(n_warm, n_out)
    in_maps = _prep_core_inputs(inputs, Wk, Wr, b, Wd, bd, n_warm, n_out)
    res = run_bass_kernel_spmd(nc, in_maps, core_ids=list(range(N_CORES)))

    out = np.empty((B_FULL, n_out, F_DIM), np.float32)
    for c in range(N_CORES):
        o = res.results[c]["out"].reshape(F_DIM, n_out, B)       # [F, t, b]
        out[c * B:(c + 1) * B] = o.transpose(2, 1, 0)
    return out
